# revision 53
# baseline (speedup 1.0000x reference)
"""Causal self-attention (B=4, T=2048, C=1024, H=16) on 8 Trainium2 NeuronCores.

Sharding: core = (batch b = core//2, head-group g = core%2, 8 heads each).

Single fused pass per core, loop over four 512-token slices nt:
  - QKV projection of slice nt (f32r matmuls, full PE rate). Q/K biases
    fold into the PSUM evacuation; the 1/sqrt(D) q-scale is folded into
    Wq/bq host-side. V' is stored bf16 WITHOUT its bias (y = y_hat + bv,
    so bv@Wproj folds into bproj host-side), with a ones*pad 65th column
    so PV's row 64 accumulates the softmax denominator (padding mask
    folded into V' at zero per-tile cost).
  - Attention q-block qt=nt immediately after (exp on ScalarE overlaps
    the next slice's projection matmuls). S^T = K^T.T @ Q^T row-tiled per
    head pair; the causal tril mask is applied INSIDE PSUM via an extra
    bf16 matmul per head on diagonal k-tiles (tril128 stationary x
    shifted -1e30 identity moving, start=False accumulate), so no DVE op
    sits on the S->exp->PV critical path. Diagonal pieces clip to
    q0=off*128 (real HW has no small-free-dim f32r penalty).
  - exp writes P bf16; PV runs all-bf16 into an f32 PSUM accumulator.
  - Normalize: DVE copy of PSUM row 64 to partition 0 (custom-DVE ops
    must not read odd PSUM partitions), reciprocal_approx_fast, gpsimd
    partition-broadcast, DVE multiplies into bf16 y^T.
  - Output projection (bf16 y^T stationary, bf16 Wproj moving) lags one
    q-block so its matmuls fill PE time while normalizes drain; partial
    outputs are written bf16 and summed host-side.
Host: transposes x per batch, slices Wqkv/Wproj by head group, pre-scales
Wq/bq by 0.125, builds the tril/mask-moving constants, sums the two bf16
partials per batch and adds bproj + bv@Wproj in f32.
"""

import os
import sys

for _p in ("/opt/trn_rl_repo",):
    if _p not in sys.path:
        sys.path.append(_p)

import numpy as np
import ml_dtypes

B, T, C = 4, 2048, 1024
H, D = 16, 64
HPC = 8          # heads per core
GC = HPC * D     # 512 channels per core
N_CORES = 8
P = 128
NT = T // 512    # 4  q-blocks / n-slices of 512
MT = GC // 128   # 4  m-tiles (head pairs)
CT = C // 128    # 8  contraction tiles
TT = T // 128    # 16 t-tiles of 128
NEG = -1.0e30

_cached = {}


def _build():
    import concourse.tile as tile
    from concourse import bacc, mybir
    import concourse.bass as bass

    f32 = mybir.dt.float32
    f32r = mybir.dt.float32r
    bf16 = mybir.dt.bfloat16
    AF = mybir.ActivationFunctionType
    ADD = mybir.AluOpType.add
    MUL = mybir.AluOpType.mult

    nc = bacc.Bacc("TRN2", target_bir_lowering=False, debug=False)

    xT_d = nc.dram_tensor("xT", [C, T], f32, kind="ExternalInput")
    wq_d = nc.dram_tensor("wq", [C, GC], f32, kind="ExternalInput")
    wk_d = nc.dram_tensor("wk", [C, GC], f32, kind="ExternalInput")
    wv_d = nc.dram_tensor("wv", [C, GC], f32, kind="ExternalInput")
    bq_d = nc.dram_tensor("bq", [GC], f32, kind="ExternalInput")
    bk_d = nc.dram_tensor("bk", [GC], f32, kind="ExternalInput")
    wp_d = nc.dram_tensor("wp", [GC, C], bf16, kind="ExternalInput")
    pad_d = nc.dram_tensor("pad", [T], f32, kind="ExternalInput")
    tril_d = nc.dram_tensor("tril", [P, P], bf16, kind="ExternalInput")
    mm_d = nc.dram_tensor("mmov", [P, 2 * P], bf16, kind="ExternalInput")
    out_d = nc.dram_tensor("out", [T, C], bf16, kind="ExternalOutput")

    with tile.TileContext(nc) as tc:
        with tc.tile_pool(name="persist", bufs=1) as persist, \
             tc.tile_pool(name="allps", bufs=2, space="PSUM") as allps, \
             tc.tile_pool(name="wpool", bufs=1) as wpool, \
             tc.tile_pool(name="xpool", bufs=2) as xpool, \
             tc.tile_pool(name="ypool", bufs=2) as ypool, \
             tc.tile_pool(name="ppool", bufs=5) as ppool, \
             tc.tile_pool(name="lpool", bufs=1) as lpool, \
             tc.tile_pool(name="bpool", bufs=2) as bpool, \
             tc.tile_pool(name="prpool", bufs=3) as prpool:
            QT = persist.tile([P, MT, T], f32r, tag="QT")
            KT = persist.tile([P, MT, T], f32r, tag="KT")
            Vp = persist.tile([P, TT, HPC, D + 1], bf16, tag="Vp")
            pad_s = persist.tile([P, TT], f32, tag="pad")
            bq_s = persist.tile([P, MT], f32, tag="bq")
            bk_s = persist.tile([P, MT], f32, tag="bk")
            tril_s = persist.tile([P, P], bf16, tag="tril")
            mmov_s = persist.tile([P, 2 * P], bf16, tag="mmov")

            wv_s = wpool.tile([P, CT, GC], f32r, tag="wv")
            wk_s = wpool.tile([P, CT, GC], f32r, tag="wk")
            wq_s = wpool.tile([P, CT, GC], f32r, tag="wq")
            wp_s = wpool.tile([P, MT, C], bf16, tag="wp")

            xTr = xT_d.rearrange("(c p) t -> p c t", p=P).bitcast(f32r)
            wvr = wv_d.rearrange("(c p) n -> p c n", p=P).bitcast(f32r)
            wkr = wk_d.rearrange("(c p) n -> p c n", p=P).bitcast(f32r)
            wqr = wq_d.rearrange("(c p) n -> p c n", p=P).bitcast(f32r)

            # startup: interleave x/wv chunks at 1-c granularity so the V
            # c-loop's matmul k can start as soon as chunk k has landed
            xt0 = xpool.tile([P, CT, 512], f32r, tag="xt")
            for c in range(CT):
                nc.sync.dma_start(xt0[:, c:c + 1, :], xTr[:, c:c + 1, 0:512])
                nc.sync.dma_start(wv_s[:, c:c + 1, :], wvr[:, c:c + 1, :])
            nc.sync.dma_start(pad_s[:], pad_d.rearrange("(tt p) -> p tt", p=P))
            nc.sync.dma_start(tril_s[:], tril_d[:])
            nc.sync.dma_start(mmov_s[:], mm_d[:])
            nc.sync.dma_start(bq_s[:], bq_d.rearrange("(m p) -> p m", p=P))
            nc.sync.dma_start(bk_s[:], bk_d.rearrange("(m p) -> p m", p=P))
            for c2 in range(0, CT, 2):
                nc.sync.dma_start(wk_s[:, c2:c2 + 2, :], wkr[:, c2:c2 + 2, :])
            for c2 in range(0, CT, 2):
                nc.sync.dma_start(wq_s[:, c2:c2 + 2, :], wqr[:, c2:c2 + 2, :])
            # Vp pad column: Vp[:, tt, h, 64] = pad[tt*128 + p] for all h.
            # On gpsimd (SBUF-only op) so the DVE queue stays clear for the
            # first V/K/Q PSUM evacuations.
            for tt in range(TT):
                nc.gpsimd.memset(Vp[:, tt, :, D:D + 1], 1.0)
                nc.gpsimd.tensor_scalar(
                    out=Vp[:, tt, :, D:D + 1], in0=Vp[:, tt, :, D:D + 1],
                    scalar1=pad_s[:, tt:tt + 1], scalar2=None, op0=MUL)

            def proj_v(nt, xt_n):
                # V bias is NOT applied here: y = sum_k P*pad*(V+bv)/l
                # = y_hat + bv, so bv@Wproj is folded into bproj host-side
                for ts in range(4):
                    tt = nt * 4 + ts
                    ps = allps.tile([P, GC], f32, tag="SS")
                    for c in range(CT):
                        nc.tensor.matmul(
                            ps[:], xt_n[:, c, ts * P:(ts + 1) * P], wv_s[:, c, :],
                            start=(c == 0), stop=(c == CT - 1))
                    nc.vector.tensor_scalar(
                        out=Vp[:, tt, :, 0:D],
                        in0=ps[:].rearrange("p (h d) -> p h d", h=HPC),
                        scalar1=pad_s[:, tt:tt + 1], scalar2=None, op0=MUL)

            def proj_kq(nt, xt_n):
                for W, bias, OUT in ((wk_s, bk_s, KT), (wq_s, bq_s, QT)):
                    for m in range(MT):
                        ps = allps.tile([P, 512], f32, tag="SS")
                        for c in range(CT):
                            nc.tensor.matmul(
                                ps[:], W[:, c, m * P:(m + 1) * P], xt_n[:, c, :],
                                start=(c == 0), stop=(c == CT - 1))
                        nc.vector.tensor_scalar(
                            out=OUT[:, m, nt * 512:(nt + 1) * 512], in0=ps[:],
                            scalar1=bias[:, m:m + 1], scalar2=None, op0=ADD)

            def proj_piece(qt_, yT_, ts, final=False):
                tt = qt_ * 4 + ts
                for nh in range(2):
                    ps = allps.tile([P, 512], f32, tag="OO")
                    for cj in range(MT):
                        nc.tensor.matmul(
                            ps[:], yT_[:, cj, ts * P:(ts + 1) * P],
                            wp_s[:, cj, nh * 512:(nh + 1) * 512],
                            start=(cj == 0), stop=(cj == MT - 1))
                    ot = prpool.tile([P, 512], bf16, tag="ot")
                    if final:
                        # ScalarE is idle once the last exp has drained;
                        # keep DVE free for the last normalize's multiplies
                        nc.scalar.activation(ot[:], ps[:], AF.Copy)
                    else:
                        nc.vector.tensor_copy(ot[:], ps[:])
                    nc.sync.dma_start(
                        out_d[tt * P:(tt + 1) * P, nh * 512:(nh + 1) * 512], ot[:])

            def attention(qt, yTq, yTq_prev, v_hook=None):
                nk = 4 * (qt + 1)
                OO_map = {}
                pend = []
                LAG = 3

                def normalize_and_aux(j_):
                    OO_ = OO_map[j_]
                    tail = (qt == NT - 1 and j_ == MT - 1)
                    if tail and yTq_prev is not None:
                        # feed PE the lagged proj piece while the normalize
                        # chain (copy+recip on DVE) runs; its evac goes to
                        # the now-idle ScalarE so the chain isn't queued
                        # behind it on DVE
                        proj_piece(qt - 1, yTq_prev, j_, final=True)
                    lraw = lpool.tile([1, 2, 512], f32, tag="lraw")
                    nc.vector.tensor_copy(lraw[0:1, :, :], OO_[D:D + 1, :, :])
                    lrec = lpool.tile([1, 2, 512], f32, tag="lrec")
                    nc.vector.reciprocal_approx_fast(
                        lrec[0:1, :, :], lraw[0:1, :, :])
                    bc = bpool.tile([P, 2, 512], f32, tag="bc")
                    nc.gpsimd.partition_broadcast(
                        bc[:], lrec[0:1, :, :], channels=P)
                    nc.vector.tensor_mul(yTq[0:D, j_, :], OO_[0:D, 0, :], bc[0:D, 0, :])
                    nc.vector.tensor_mul(yTq[D:P, j_, :], OO_[0:D, 1, :], bc[D:P, 1, :])
                    if not tail and yTq_prev is not None:
                        proj_piece(qt - 1, yTq_prev, j_)

                def emit_pv(entry):
                    j_, k_, z_, PP_ = entry
                    OO_ = OO_map[j_]
                    last = (k_ == nk - 1)
                    for e in range(2):
                        nc.tensor.matmul(
                            OO_[:, e, z_:512], Vp[:, k_, 2 * j_ + e, :],
                            PP_[:, e, z_:512],
                            start=(k_ == 0), stop=last)
                    if last:
                        normalize_and_aux(j_)

                for j in range(MT):
                    OO_map[j] = allps.tile(
                        [D + 1, 2, 512], f32, tag="OO", name="OO")
                    for kt in range(nk):
                        off = kt - 4 * qt
                        diag = off >= 0
                        q0 = off * P if diag else 0
                        SS = allps.tile([P, 2, 512], f32, tag="SS")
                        for e, lo, hi in ((0, 0, D), (1, D, P)):
                            nc.tensor.matmul(
                                SS[:, e, q0:512], KT[lo:hi, j, kt * P:(kt + 1) * P],
                                QT[lo:hi, j, qt * 512 + q0:(qt + 1) * 512],
                                start=True, stop=True, skip_group_check=True)
                        if diag:
                            # one mask matmul covers both heads: the strided
                            # out AP iterates (e, col) and mmov holds the
                            # shifted -1e30 triangle twice
                            nc.tensor.matmul(
                                SS[:, :, q0:q0 + P], tril_s[:],
                                mmov_s[:, 0:2 * P],
                                start=False, stop=True, skip_group_check=True)
                        PP = ppool.tile([P, 2, 512], bf16, tag="PP")
                        nc.scalar.activation(
                            PP[:, :, q0:512], SS[:, :, q0:512], AF.Exp)
                        pend.append((j, kt, q0, PP))
                        if len(pend) > LAG:
                            emit_pv(pend.pop(0))
                if v_hook is not None:
                    # all S-issues done: the next slice's V projection fills
                    # PE while the pend-drain PVs and normalizes finish
                    v_hook()
                while pend:
                    emit_pv(pend.pop(0))

            xt_n = xt0
            yTq_prev = None
            wpr = wp_d.rearrange("(m p) n -> p m n", p=P)
            for nt in range(NT):
                if nt + 1 < NT:
                    xt_next = xpool.tile([P, CT, 512], f32r, tag="xt")
                    nc.sync.dma_start(
                        xt_next[:], xTr[:, :, (nt + 1) * 512:(nt + 2) * 512])
                else:
                    xt_next = None
                if nt == 1:
                    # wp is first read during attention(1)'s lagged proj;
                    # deferring its 1 MB off the startup window keeps the
                    # early DMA bandwidth for x/wv/wk/wq
                    for m in range(MT):
                        nc.sync.dma_start(wp_s[:, m, :], wpr[:, m, :])
                if nt == 0:
                    proj_v(0, xt_n)
                proj_kq(nt, xt_n)
                yTq = ypool.tile([P, MT, 512], bf16, tag="yTq")
                v_hook = None
                if xt_next is not None:
                    v_hook = (lambda nt=nt, xt=xt_next: proj_v(nt + 1, xt))
                attention(nt, yTq, yTq_prev, v_hook=v_hook)
                yTq_prev = yTq
                xt_n = xt_next
            for ts in range(4):
                proj_piece(NT - 1, yTq_prev, ts, final=True)

    nc.compile()
    return nc


def _get_nc():
    if "nc" not in _cached:
        _cached["nc"] = _build()
    return _cached["nc"]


def kernel(x, padding_mask, Wqkv, bqkv, Wproj, bproj):
    from concourse.bass_utils import run_bass_kernel_spmd

    x = np.asarray(x, dtype=np.float32)
    padding_mask = np.asarray(padding_mask)
    Wqkv = np.asarray(Wqkv, dtype=np.float32)
    bqkv = np.asarray(bqkv, dtype=np.float32)
    Wproj = np.asarray(Wproj, dtype=np.float32)
    bproj = np.asarray(bproj, dtype=np.float32)
    assert x.shape == (B, T, C), x.shape

    nc = _get_nc()
    bf = ml_dtypes.bfloat16
    cc = np.arange(P)[:, None]
    pp = np.arange(P)[None, :]
    tril = (cc <= pp).astype(np.float32).astype(bf)
    mmov = np.zeros((P, 2 * P), dtype=np.float32)
    for m_ in range(P - 1):
        mmov[m_ + 1, m_] = NEG
        mmov[m_ + 1, P + m_] = NEG
    mmov = mmov.astype(bf)

    in_maps = []
    for core in range(N_CORES):
        b, g = divmod(core, 2)
        sl = slice(g * GC, (g + 1) * GC)
        in_maps.append({
            "xT": np.ascontiguousarray(x[b].T),
            "wq": np.ascontiguousarray(Wqkv[:, 0 * C:1 * C][:, sl]) * 0.125,
            "wk": np.ascontiguousarray(Wqkv[:, 1 * C:2 * C][:, sl]),
            "wv": np.ascontiguousarray(Wqkv[:, 2 * C:3 * C][:, sl]),
            "bq": np.ascontiguousarray(bqkv[0 * C:1 * C][sl]) * 0.125,
            "bk": np.ascontiguousarray(bqkv[1 * C:2 * C][sl]),
            "wp": np.ascontiguousarray(Wproj[g * GC:(g + 1) * GC, :]).astype(bf),
            "pad": padding_mask[b].astype(np.float32),
            "tril": tril,
            "mmov": mmov,
        })

    trace = bool(os.environ.get("BASS_KERNEL_TRACE"))
    res = run_bass_kernel_spmd(
        nc, in_maps, core_ids=list(range(N_CORES)), trace=trace)
    _cached["last_result"] = res

    # V bias is excluded from V' in-kernel; y = y_hat + bv, so its
    # projection bv @ Wproj folds into the output bias
    bproj_eff = bproj + bqkv[2 * C:3 * C] @ Wproj

    out = np.empty((B, T, C), dtype=np.float32)
    for b in range(B):
        out[b] = (res.results[2 * b]["out"].astype(np.float32)
                  + res.results[2 * b + 1]["out"].astype(np.float32) + bproj_eff)
    return out
